# revision 1
# baseline (speedup 1.0000x reference)
"""Multi-head causal attention (B=4, S=2048, D=1024, H=16, HD=64) on 8 TRN2 cores.

Sharding: core c handles (batch b = c//2, head-group hg = c%2 of 8 heads).
Each core computes QKV projections for its 512-dim head slice, transposed-layout
causal attention, and a partial output projection. Host sums the two head-group
partials per batch and adds the bias.

Per-core pipeline (all matmuls plain — no tile_position, verified on HW):
  - x fed pre-transposed: xT [1024, 2048]; QT/KT/V via f32r K=128 matmuls.
  - KT stored bf16 in block-diagonal pair tiles [128, 2S]: head-even data on
    partitions 0-63 (cols 0:S), head-odd on partitions 64-127 (cols S:2S),
    zeros elsewhere, so transposed scores ST[k, q] = K_chunk @ Q^T run as full
    K=128 bf16 matmuls per head with the shared stacked QT as rhs.
  - P^T = exp(ST/8) on ScalarE (f32r out); causal masking via multiplicative
    triangular masks on DVE; ragged column windows clamped to >=256.
  - ctx^T = V_aug^T @ P^T with V_aug = [V_h | ones] (M=65, f32r): row 64
    accumulates the softmax denominators r[q] for free.
  - r rows moved to partition 0 by SBUF-SBUF DMA, broadcast across partitions
    by K=1 f32r matmuls, reciprocal + normalize on DVE; head-odd ctx rows
    moved to partitions 64-127 by DMA to form paired ctxT tiles.
  - out[s, :] partial = sum_pairs ctxT_pair.T @ Wo_pair (f32r K=128).
"""

import os
import numpy as np

import concourse.bass as bass
import concourse.mybir as mybir
from concourse import bacc
import concourse.tile as tile
from concourse.bass_utils import run_bass_kernel_spmd

F32 = mybir.dt.float32
F32R = mybir.dt.float32r
BF16 = mybir.dt.bfloat16
EXP = mybir.ActivationFunctionType.Exp

P = 128
S = 2048
DIN = 1024
DH = 512          # per-core d_out slice (8 heads x 64)
NKC = DIN // P    # 8 contraction chunks
NPAIR = 4         # head pairs per core
NPIECE = S // 512 # 4 q pieces
W = 512

# ragged start offsets for diagonal chunks (krel = chunk - 4*piece); widths
# W - sk are clamped >= 256 so f32r matmuls stay at full rate.
SKS = [0, 128, 256, 256]


def build_program(repeat: int = 1) -> bass.Bass:
    nc = bacc.Bacc("TRN2", target_bir_lowering=False)

    xT_d = nc.dram_tensor("xT", [DIN, S], BF16, kind="ExternalInput")
    wq_d = nc.dram_tensor("wq", [DIN, DH], BF16, kind="ExternalInput")
    wk_d = nc.dram_tensor("wk", [DIN, DH], BF16, kind="ExternalInput")
    wv_d = nc.dram_tensor("wv", [DIN, DH], BF16, kind="ExternalInput")
    wo_d = nc.dram_tensor("wo", [DH, DIN], F32, kind="ExternalInput")
    maskA_d = nc.dram_tensor("maskA", [P, P], BF16, kind="ExternalInput")
    maskB_d = nc.dram_tensor("maskB", [P, 2 * P], BF16, kind="ExternalInput")
    ones_d = nc.dram_tensor("ones", [P, 64], F32, kind="ExternalInput")
    e64_d = nc.dram_tensor("e64", [P, 65], F32, kind="ExternalInput")
    out_d = nc.dram_tensor("out", [S, DIN], F32, kind="ExternalOutput")

    with tile.TileContext(nc) as tc:
        with (
            tc.tile_pool(name="consts", bufs=1) as consts,
            tc.tile_pool(name="xtp", bufs=2) as xtp,
            tc.tile_pool(name="qtp", bufs=3) as qtp,
            tc.tile_pool(name="ptp", bufs=4) as ptp,
            tc.tile_pool(name="ctxtp", bufs=3) as ctxtp,
            tc.tile_pool(name="rp", bufs=1) as rp,
            tc.tile_pool(name="osbp", bufs=3) as osbp,
            tc.tile_pool(name="ps_st", bufs=2, space="PSUM") as ps_st,
            tc.tile_pool(name="ps_ctx", bufs=1, space="PSUM") as ps_ctx,
            tc.tile_pool(name="ps_mm", bufs=2, space="PSUM") as ps_mm,
        ):
            # ---- prefetch first xT piece before weights ----
            xT_r0 = xT_d.rearrange("(kc p) s -> p kc s", p=P)
            xt0 = xtp.tile([P, NKC, W], BF16, tag="xt", name="xt0")
            for kc in range(NKC):
                nc.sync.dma_start(
                    xt0[:, kc, :], xT_r0[:, kc, 0:W]
                )

            # ---- constants / weights ----
            wq_sb = consts.tile([P, NKC, DH], BF16)
            wk_sb = consts.tile([P, NKC, DH], BF16)
            wv_sb = consts.tile([P, NKC, DH], BF16)
            wo_sb = consts.tile([P, NPAIR, DIN], F32R)
            maskA = consts.tile([P, P], BF16)
            maskB = consts.tile([P, 2 * P], BF16)
            ones_row = consts.tile([P, 64], F32R)
            wq_r = wq_d.rearrange("(kc p) d -> p kc d", p=P)
            for kc in range(NKC):
                nc.sync.dma_start(wq_sb[:, kc, :], wq_r[:, kc, :])
            wk_r = wk_d.rearrange("(kc p) d -> p kc d", p=P)
            for half in range(2):
                nc.sync.dma_start(
                    wk_sb[:, 4 * half : 4 * half + 4, :],
                    wk_r[:, 4 * half : 4 * half + 4, :],
                )
            wv_r = wv_d.rearrange("(kc p) d -> p kc d", p=P)
            for half in range(2):
                nc.sync.dma_start(
                    wv_sb[:, 4 * half : 4 * half + 4, :],
                    wv_r[:, 4 * half : 4 * half + 4, :],
                )
            nc.sync.dma_start(wo_sb[:], wo_d.rearrange("(g p) d -> p g d", p=P).bitcast(F32R))
            nc.sync.dma_start(maskA[:], maskA_d[:])
            nc.sync.dma_start(maskB[:], maskB_d[:])
            nc.sync.dma_start(ones_row[:], ones_d[:].bitcast(F32R))
            e64_sb = consts.tile([P, 65], F32R)
            nc.sync.dma_start(e64_sb[:], e64_d[:].bitcast(F32R))

            # K^T per pair, bf16 block-diagonal [128, 2S]; V per piece
            # [128, s-chunk(4), head(8), 65] with ones in column 64.
            kt_sb = [consts.tile([P, 2 * S], BF16, name=f"kt{j}") for j in range(NPAIR)]
            v_sb = [
                consts.tile([P, 4, 8, 65], BF16, name=f"v{pp}") for pp in range(NPIECE)
            ]
            for j in range(NPAIR):
                nc.vector.memset(kt_sb[j][:], 0.0)
            for pp in range(NPIECE):
                nc.vector.memset(v_sb[pp][:, :, :, 64], 1.0)

            xT_r = xT_d.rearrange("(kc p) s -> p kc s", p=P)

            for _rep in range(repeat):
              for p in range(NPIECE):
                  scol = W * p
                  # ---- load xT piece ----
                  if p == 0 and _rep == 0:
                      xt = xt0
                  else:
                      xt = xtp.tile([P, NKC, W], BF16, tag="xt")
                      for kc in range(NKC):
                          nc.sync.dma_start(
                              xt[:, kc, :],
                              xT_r[:, kc, scol : scol + W],
                          )

                  # ---- QT per pair (bf16) ----
                  qts = []
                  for j in range(NPAIR):
                      ps = ps_mm.tile([P, W], F32, tag="mm")
                      for kc in range(NKC):
                          nc.tensor.matmul(
                              ps[:],
                              wq_sb[:, kc, P * j : P * j + P],
                              xt[:, kc, :],
                              start=(kc == 0),
                              stop=(kc == NKC - 1),
                          )
                      qt = qtp.tile([P, W], BF16, tag=f"qt{j}", name=f"qt{j}_{p}")
                      nc.scalar.copy(qt[:], ps[:])
                      qts.append(qt)

                  # ---- KT per pair into bf16 block-diagonal tiles ----
                  for j in range(NPAIR):
                      ps = ps_mm.tile([P, W], F32, tag="mm")
                      for kc in range(NKC):
                          nc.tensor.matmul(
                              ps[:],
                              wk_sb[:, kc, P * j : P * j + P],
                              xt[:, kc, :],
                              start=(kc == 0),
                              stop=(kc == NKC - 1),
                          )
                      nc.vector.tensor_copy(
                          kt_sb[j][0:64, scol : scol + W], ps[0:64, :]
                      )
                      nc.vector.tensor_copy(
                          kt_sb[j][64:128, S + scol : S + scol + W], ps[64:128, :]
                      )

                  # ---- V per s-chunk ----
                  for i in range(4):
                      ps = ps_mm.tile([P, W], F32, tag="mm")
                      for kc in range(NKC):
                          nc.tensor.matmul(
                              ps[:],
                              xt[:, kc, P * i : P * i + P],
                              wv_sb[:, kc, :],
                              start=(kc == 0),
                              stop=(kc == NKC - 1),
                          )
                      nc.vector.tensor_copy(
                          v_sb[p][:, i, :, 0:64],
                          ps[:].rearrange("q (h d) -> q h d", h=8),
                      )

                  # ---- attention per pair ----
                  nch = 4 * p + 4  # chunks 0..4p+3
                  for j in range(NPAIR):
                      ctx = ps_ctx.tile([P, 2 * W], F32, tag="ctx", name=f"ctx{p}_{j}")
                      for c in range(nch):
                          krel = c - 4 * p
                          sk = SKS[krel] if krel >= 0 else 0
                          st = ps_st.tile([P, 2 * W], F32, tag="st", name=f"st{p}_{j}_{c}")
                          for hl in range(2):
                              nc.tensor.matmul(
                                  st[:, W * hl + sk : W * hl + W],
                                  kt_sb[j][:, S * hl + P * c : S * hl + P * c + P],
                                  qts[j][:, sk:W],
                                  start=True,
                                  stop=True,
                              )
                          pt = ptp.tile([P, 2 * W], BF16, tag="pt", name=f"pt{p}_{j}_{c}")
                          st3 = st[:].rearrange("q (h n) -> q h n", h=2)
                          pt3 = pt[:].rearrange("q (h n) -> q h n", h=2)
                          nc.scalar.activation(
                              pt3[:, :, sk:W], st3[:, :, sk:W], EXP, scale=0.125
                          )
                          if krel >= 0:
                              if krel <= 2:
                                  nc.vector.tensor_mul(
                                      pt3[:, :, sk : sk + P],
                                      pt3[:, :, sk : sk + P],
                                      maskA[:, None, :].broadcast_to([P, 2, P]),
                                  )
                              else:
                                  nc.vector.tensor_mul(
                                      pt3[:, :, 2 * P : W],
                                      pt3[:, :, 2 * P : W],
                                      maskB[:, None, :].broadcast_to([P, 2, 2 * P]),
                                  )
                          for hl in range(2):
                              h = 2 * j + hl
                              nc.tensor.matmul(
                                  ctx[0:65, W * hl + sk : W * hl + W],
                                  v_sb[c // 4][:, c % 4, h, :],
                                  pt[:, W * hl + sk : W * hl + W],
                                  start=(c == 0),
                                  stop=(c == nch - 1),
                                  skip_group_check=True,
                              )

                      # ---- normalize: r row -> sbuf, reciprocal (1-lane),
                      # move+broadcast via e64 selector matmuls, multiply ----
                      craw = rp.tile([65, 2 * W], F32R, tag="craw", name=f"cr{p}_{j}", bufs=2)
                      nc.vector.tensor_copy(craw[:], ctx[0:65, :])
                      bcs = []
                      for hl in range(2):
                          bc = ps_mm.tile([P, W], F32, tag="mm", name=f"bc{p}_{j}_{hl}")
                          nc.tensor.matmul(
                              bc[0:65, :],
                              e64_sb[0:65, :],
                              craw[0:65, W * hl : W * hl + W],
                              start=True,
                              stop=True,
                              skip_group_check=True,
                          )
                          bcs.append(bc)
                      rbr = rp.tile([64, 2 * W], F32, tag="rbr", name=f"rbr{p}_{j}", bufs=2)
                      nc.vector.reciprocal(rbr[:, 0:W], bcs[0][0:64, :])
                      nc.vector.reciprocal(rbr[:, W : 2 * W], bcs[1][0:64, :])
                      ctxt = ctxtp.tile([P, W], F32R, tag=f"ctxt{j}", name=f"ctxt{j}_{p}")
                      nc.vector.tensor_mul(
                          ctxt[0:64, :], craw[0:64, 0:W], rbr[:, 0:W]
                      )
                      hstage = rp.tile([64, W], F32R, tag="hs", name=f"hs{p}_{j}", bufs=2)
                      nc.vector.tensor_mul(
                          hstage[:], craw[0:64, W : 2 * W], rbr[:, W : 2 * W]
                      )
                      nc.sync.dma_start(ctxt[64:128, 0:256], hstage[:, 0:256])
                      nc.sync.dma_start(ctxt[64:128, 256:W], hstage[:, 256:W])
                      if j == 0:
                          ctxts = [None] * NPAIR
                      ctxts[j] = ctxt

                  # ---- output projection for this piece ----
                  for si in range(4):
                      for nsl in range(2):
                          ps = ps_mm.tile([P, W], F32, tag="mm")
                          for g in range(NPAIR):
                              nc.tensor.matmul(
                                  ps[:],
                                  ctxts[g][:, P * si : P * si + P],
                                  wo_sb[:, g, W * nsl : W * nsl + W],
                                  start=(g == 0),
                                  stop=(g == NPAIR - 1),
                              )
                          osb = osbp.tile([P, W], F32, tag="osb")
                          nc.vector.tensor_copy(osb[:], ps[:])
                          nc.sync.dma_start(
                              out_d[
                                  scol + P * si : scol + P * si + P,
                                  W * nsl : W * nsl + W,
                              ],
                              osb[:],
                          )
    nc.compile()
    return nc


_program = None
last_results = None


def _get_program():
    global _program
    if _program is None:
        _program = build_program()
    return _program


def kernel(x, Wq, Wk, Wv, Wo, bo):
    global last_results
    x = np.asarray(x, dtype=np.float32)
    Wq = np.asarray(Wq, dtype=np.float32)
    Wk = np.asarray(Wk, dtype=np.float32)
    Wv = np.asarray(Wv, dtype=np.float32)
    Wo = np.asarray(Wo, dtype=np.float32)
    bo = np.asarray(bo, dtype=np.float32)

    import ml_dtypes
    maskA = np.triu(np.ones((P, P), dtype=ml_dtypes.bfloat16))
    maskB = np.concatenate([np.zeros((P, P), ml_dtypes.bfloat16), maskA], axis=1)
    ones = np.ones((P, 64), dtype=np.float32)
    e64 = np.zeros((P, 65), dtype=np.float32)
    e64[64, :] = 1.0

    nc = _get_program()
    in_maps = []
    for c in range(8):
        b, hg = c // 2, c % 2
        in_maps.append(
            {
                "xT": np.ascontiguousarray(x[b].T).astype(ml_dtypes.bfloat16),
                "wq": np.ascontiguousarray(
                    Wq[:, DH * hg : DH * hg + DH]
                ).astype(ml_dtypes.bfloat16),
                "wk": np.ascontiguousarray(
                    Wk[:, DH * hg : DH * hg + DH]
                ).astype(ml_dtypes.bfloat16),
                "wv": np.ascontiguousarray(
                    Wv[:, DH * hg : DH * hg + DH]
                ).astype(ml_dtypes.bfloat16),
                "wo": np.ascontiguousarray(Wo[DH * hg : DH * hg + DH, :]),
                "maskA": maskA,
                "maskB": maskB,
                "ones": ones,
                "e64": e64,
            }
        )
    trace = bool(os.environ.get("KERNEL_TRACE"))
    last_results = run_bass_kernel_spmd(
        nc, in_maps, core_ids=list(range(8)), trace=trace
    )
    outs = [r["out"] for r in last_results.results]
    return np.stack([outs[2 * b] + outs[2 * b + 1] + bo for b in range(4)])



# revision 15
# speedup vs baseline: 1.1867x; 1.1867x over previous
"""Multi-head causal attention (B=4, S=2048, D=1024, H=16, HD=64) on 8 TRN2 cores.

Sharding: core c handles (batch b = c//2, head-group hg = c%2 of 8 heads).
Each core computes QKV projections for its 512-dim head slice, transposed-layout
causal attention, and a partial output projection. Host sums the two head-group
partials per batch and adds the bias.

Per-core pipeline (v2 — software-pipelined across pieces):
  - x fed pre-transposed: xT [1024, 2048]; QT/KT/V via bf16 K=128 matmuls.
  - KT per pair stored bf16 [128, S]: head-even dims on partitions 0-63,
    head-odd on 64-127 (no zero padding); scores run as K=64 matmuls with
    base-partition-64 operands for the odd head (tile_position inferred).
  - Exact causal ragged windows SKS=[0,128,256,384]; a single upper-triangular
    maskA multiplies the 128-wide diagonal block of each kept chunk.
  - P^T = exp(ST/8) on ScalarE; ctx^T = V_aug^T @ P^T with V_aug = [V_h | 1]
    (M=65): row 64 accumulates softmax denominators r[q] for free.
  - normalize: r row copied to SBUF (f32r), broadcast to partitions 0-63 by a
    K=1 f32r matmul (213ns) into a st-ring PSUM tile, reciprocal + muls on
    DVE; head-odd ctx rows moved to partitions 64-127 by one SBUF-SBUF DMA
    issued via GpSimd's software DGE.
  - out[s, :] partial = sum_pairs ctxT_pair.T @ Wo_pair (K=128, moving f32r).
  - Engine schedule: QKV matmuls of piece p+1 and the output projection of
    piece p-1 are woven between attention matmuls of piece p in program order
    so TensorE never starves while ScalarE runs the exp chain.
"""

import os
import numpy as np

import concourse.bass as bass
import concourse.mybir as mybir
from concourse import bacc
import concourse.tile as tile
from concourse.bass_utils import run_bass_kernel_spmd

F32 = mybir.dt.float32
F32R = mybir.dt.float32r
BF16 = mybir.dt.bfloat16
EXP = mybir.ActivationFunctionType.Exp

P = 128
S = 2048
DIN = 1024
DH = 512          # per-core d_out slice (8 heads x 64)
NKC = DIN // P    # 8 contraction chunks
NPAIR = 4         # head pairs per core
NPIECE = S // 512 # 4 q pieces
W = 512

# exact causal ragged start offsets for diagonal chunks (krel = chunk - 4*piece)
SKS = [0, 128, 256, 384]

MM_NS = 0.4167  # PE ns/col, used only for weave pacing


def build_program() -> bass.Bass:
    nc = bacc.Bacc("TRN2", target_bir_lowering=False)

    xT_d = nc.dram_tensor("xT", [DIN, S], BF16, kind="ExternalInput")
    wq_d = nc.dram_tensor("wq", [DIN, DH], BF16, kind="ExternalInput")
    wk_d = nc.dram_tensor("wk", [DIN, DH], BF16, kind="ExternalInput")
    wv_d = nc.dram_tensor("wv", [DIN, DH], BF16, kind="ExternalInput")
    wo_d = nc.dram_tensor("wo", [DH, DIN], F32, kind="ExternalInput")
    maskA_d = nc.dram_tensor("maskA", [P, P], BF16, kind="ExternalInput")
    ones_d = nc.dram_tensor("ones", [P, 64], F32, kind="ExternalInput")
    out_d = nc.dram_tensor("out", [S, DIN], F32, kind="ExternalOutput")

    with tile.TileContext(nc) as tc:
        with (
            tc.tile_pool(name="consts", bufs=1) as consts,
            tc.tile_pool(name="xtp", bufs=4) as xtp,
            tc.tile_pool(name="qtp", bufs=2) as qtp,
            tc.tile_pool(name="ptp", bufs=4) as ptp,
            tc.tile_pool(name="ctxtp", bufs=2) as ctxtp,
            tc.tile_pool(name="rp", bufs=2) as rp,
            tc.tile_pool(name="osbp", bufs=3) as osbp,
            tc.tile_pool(name="ps_st", bufs=2, space="PSUM") as ps_st,
            tc.tile_pool(name="ps_ctx", bufs=1, space="PSUM") as ps_ctx,
            tc.tile_pool(name="ps_mm", bufs=2, space="PSUM") as ps_mm,
        ):
            # ---- input/weight DMAs, batched one per tensor, issued on SP ----
            xT_r = xT_d.rearrange("(kc p) s -> p kc s", p=P)
            xts = []
            wq_sb = consts.tile([P, NKC, DH], BF16)
            wk_sb = consts.tile([P, NKC, DH], BF16)
            wv_sb = consts.tile([P, NKC, DH], BF16)
            wo_sb = consts.tile([P, NPAIR, DIN], F32R)
            maskA = consts.tile([P, P], BF16)

            wq_r = wq_d.rearrange("(kc p) d -> p kc d", p=P)
            nc.sync.dma_start(
                wq_sb[:, :, 0:P], wq_r[:, :, 0:P]
            )
            # piece-0 x in two half-tiles so the first Q matmuls start sooner
            xt0a = xtp.tile([P, NKC // 2, W], BF16, tag="xta", name="xt0a")
            xt0b = xtp.tile([P, NKC // 2, W], BF16, tag="xtb", name="xt0b")
            nc.sync.dma_start(xt0a[:], xT_r[:, 0 : NKC // 2, 0:W])
            nc.sync.dma_start(xt0b[:], xT_r[:, NKC // 2 : NKC, 0:W])
            xts.append((xt0a, xt0b))
            for j in range(1, NPAIR):
                nc.sync.dma_start(
                    wq_sb[:, :, P * j : P * j + P], wq_r[:, :, P * j : P * j + P]
                )
            nc.sync.dma_start(wv_sb[:], wv_d.rearrange("(kc p) d -> p kc d", p=P))
            nc.sync.dma_start(wk_sb[:], wk_d.rearrange("(kc p) d -> p kc d", p=P))
            for p in range(1, NPIECE):
                xa = xtp.tile([P, NKC // 2, W], BF16, tag="xta", name=f"xt{p}a")
                xb = xtp.tile([P, NKC // 2, W], BF16, tag="xtb", name=f"xt{p}b")
                nc.sync.dma_start(xa[:], xT_r[:, 0 : NKC // 2, W * p : W * p + W])
                nc.sync.dma_start(xb[:], xT_r[:, NKC // 2 : NKC, W * p : W * p + W])
                xts.append((xa, xb))
            nc.sync.dma_start(
                wo_sb[:], wo_d.rearrange("(g p) d -> p g d", p=P).bitcast(F32R)
            )
            nc.sync.dma_start(maskA[:], maskA_d[:])
            ones_r = consts.tile([P, 64], F32R)
            nc.sync.dma_start(ones_r[:], ones_d[:].bitcast(F32R))

            def xt_ap(q, kc):
                half, k = divmod(kc, NKC // 2)
                return xts[q][half][:, k, :]

            # KT per pair [128, S]: head-even dims on partitions 0-63,
            # head-odd on 64-127.  V per piece [128, s-chunk(4), head(8), 65]
            # with ones in column 64 (softmax denominator rides the PV matmul).
            kt_sb = [consts.tile([P, S], BF16, name=f"kt{j}") for j in range(NPAIR)]
            v_sb = [
                consts.tile([P, 4, 8, 65], BF16, name=f"v{pp}") for pp in range(NPIECE)
            ]
            for pp in range(NPIECE):
                nc.vector.memset(v_sb[pp][:, :, :, 64], 1.0)

            qts_all = [[None] * NPAIR for _ in range(NPIECE)]
            ctxts_all = [[None] * NPAIR for _ in range(NPIECE)]

            # ---------- emitters (closures for the weave) ----------
            def q_grp(q, j):
                def fn(q=q, j=j):
                    ps = ps_mm.tile([P, W], F32, tag="mm")
                    for kc in range(NKC):
                        nc.tensor.matmul(
                            ps[:],
                            wq_sb[:, kc, P * j : P * j + P],
                            xt_ap(q, kc),
                            start=(kc == 0),
                            stop=(kc == NKC - 1),
                        )
                    qt = qtp.tile([P, W], BF16, tag=f"qt{j}", name=f"qt{j}_{q}")
                    nc.scalar.copy(qt[:], ps[:])
                    qts_all[q][j] = qt
                return (NKC * W * MM_NS, fn)

            def k_grp(q, j):
                def fn(q=q, j=j):
                    ps = ps_mm.tile([P, W], F32, tag="mm")
                    for kc in range(NKC):
                        nc.tensor.matmul(
                            ps[:],
                            wk_sb[:, kc, P * j : P * j + P],
                            xt_ap(q, kc),
                            start=(kc == 0),
                            stop=(kc == NKC - 1),
                        )
                    nc.vector.tensor_copy(
                        kt_sb[j][:, W * q : W * q + W], ps[:]
                    )
                return (NKC * W * MM_NS, fn)

            def v_grp(q, i):
                def fn(q=q, i=i):
                    ps = ps_mm.tile([P, W], F32, tag="mm")
                    for kc in range(NKC):
                        nc.tensor.matmul(
                            ps[:],
                            xt_ap(q, kc)[:, P * i : P * i + P],
                            wv_sb[:, kc, :],
                            start=(kc == 0),
                            stop=(kc == NKC - 1),
                        )
                    nc.vector.tensor_copy(
                        v_sb[q][:, i, :, 0:64],
                        ps[:].rearrange("q (h d) -> q h d", h=8),
                    )
                return (NKC * W * MM_NS, fn)

            def op_grp(p, si, nsl):
                def fn(p=p, si=si, nsl=nsl):
                    ps = ps_mm.tile([P, W], F32, tag="mm")
                    for g in range(NPAIR):
                        nc.tensor.matmul(
                            ps[:],
                            ctxts_all[p][g][:, P * si : P * si + P],
                            wo_sb[:, g, W * nsl : W * nsl + W],
                            start=(g == 0),
                            stop=(g == NPAIR - 1),
                        )
                    osb = osbp.tile([P, W], F32, tag="osb")
                    nc.vector.tensor_copy(osb[:], ps[:])
                    nc.sync.dma_start(
                        out_d[
                            W * p + P * si : W * p + P * si + P,
                            W * nsl : W * nsl + W,
                        ],
                        osb[:],
                    )
                return (NPAIR * W * MM_NS, fn)

            def qkv_filler(q):
                # K groups last: their kt writes WAR-wait on the previous
                # piece's score reads, so land them late in the bundle.
                ops = [q_grp(q, j) for j in range(NPAIR)]
                ops += [v_grp(q, i) for i in range(4)]
                ops += [k_grp(q, j) for j in range(NPAIR)]
                return ops

            def op_filler(p):
                return [op_grp(p, si, nsl) for si in range(4) for nsl in range(2)]

            # ---------- attention primary stream for piece p ----------
            def attn_ops(p):
                ops = []
                nch = 4 * p + 4
                for j in range(NPAIR):
                    ctx_holder = {}
                    pts = {}

                    def mk_sc(p=p, j=j, c=0, pts=pts):
                        def fn(p=p, j=j, c=c, pts=pts):
                            krel = c - 4 * p
                            sk = SKS[krel] if krel >= 0 else 0
                            st = ps_st.tile(
                                [P, 2 * W], F32, tag="st", name=f"st{p}_{j}_{c}"
                            )
                            for hl in range(2):
                                nc.tensor.matmul(
                                    st[:, W * hl + sk : W * hl + W],
                                    kt_sb[j][64 * hl : 64 * hl + 64, P * c : P * c + P],
                                    qts_all[p][j][64 * hl : 64 * hl + 64, sk:W],
                                    start=True,
                                    stop=True,
                                )
                            pt = ptp.tile(
                                [P, 2 * W], BF16, tag="pt", name=f"pt{p}_{j}_{c}"
                            )
                            st3 = st[:].rearrange("q (h n) -> q h n", h=2)
                            pt3 = pt[:].rearrange("q (h n) -> q h n", h=2)
                            nc.scalar.activation(
                                pt3[:, :, sk:W], st3[:, :, sk:W], EXP, scale=0.125
                            )
                            if krel >= 0:
                                nc.vector.tensor_mul(
                                    pt3[:, :, sk : sk + P],
                                    pt3[:, :, sk : sk + P],
                                    maskA[:, None, :].broadcast_to([P, 2, P]),
                                )
                            pts[c] = pt
                        krel = c - 4 * p
                        sk = SKS[krel] if krel >= 0 else 0
                        return (2 * (W - sk) * MM_NS, fn)

                    def mk_pv(p=p, j=j, c=0, pts=pts, ctx_holder=ctx_holder, nch=nch):
                        def fn(p=p, j=j, c=c, pts=pts, ctx_holder=ctx_holder, nch=nch):
                            krel = c - 4 * p
                            sk = SKS[krel] if krel >= 0 else 0
                            if c == 0:
                                ctx_holder["ctx"] = ps_ctx.tile(
                                    [P, 2 * W], F32, tag="ctx", name=f"ctx{p}_{j}"
                                )
                            ctx = ctx_holder["ctx"]
                            pt = pts.pop(c)
                            for hl in range(2):
                                h = 2 * j + hl
                                nc.tensor.matmul(
                                    ctx[0:65, W * hl + sk : W * hl + W],
                                    v_sb[c // 4][:, c % 4, h, :],
                                    pt[:, W * hl + sk : W * hl + W],
                                    start=(c == 0),
                                    stop=(c == nch - 1),
                                    skip_group_check=True,
                                )
                        krel = c - 4 * p
                        sk = SKS[krel] if krel >= 0 else 0
                        return (2 * (W - sk) * MM_NS, fn)

                    def mk_norm(p=p, j=j, ctx_holder=ctx_holder):
                        def fn(p=p, j=j, ctx_holder=ctx_holder):
                            # r row (partition 64) -> SBUF f32r, broadcast to
                            # partitions 0-63 with a K=1 f32r matmul into a
                            # st-ring PSUM tile, reciprocal, then normalize.
                            # hl=1 first so its SBUF-SBUF move starts early.
                            ctx = ctx_holder["ctx"]
                            ctxt = ctxtp.tile(
                                [P, W], F32R, tag=f"ctxt{j}", name=f"ctxt{j}_{p}"
                            )
                            bc = ps_st.tile(
                                [P, 2 * W], F32, tag="st", name=f"bc{p}_{j}"
                            )
                            for hl in (1, 0):
                                rrow = rp.tile(
                                    [P, W], F32R, tag=f"rr{hl}", name=f"rr{hl}_{p}_{j}"
                                )
                                nc.vector.tensor_copy(
                                    rrow[64:65, :], ctx[64:65, W * hl : W * hl + W]
                                )
                                nc.tensor.matmul(
                                    bc[0:64, W * hl : W * hl + W],
                                    ones_r[64:65, :],
                                    rrow[64:65, :],
                                    start=True,
                                    stop=True,
                                    skip_group_check=True,
                                )
                                rbr = rp.tile(
                                    [64, W], F32, tag=f"rb{hl}", name=f"rb{hl}_{p}_{j}"
                                )
                                nc.vector.reciprocal(
                                    rbr[:], bc[0:64, W * hl : W * hl + W]
                                )
                                if hl == 1:
                                    hst = rp.tile(
                                        [64, W], F32R, tag="hst", name=f"hs{p}_{j}"
                                    )
                                    nc.vector.tensor_mul(
                                        hst[:], ctx[0:64, W : 2 * W], rbr[:]
                                    )
                                    nc.gpsimd.dma_start(ctxt[64:128, :], hst[:])
                                else:
                                    nc.vector.tensor_mul(
                                        ctxt[0:64, :], ctx[0:64, 0:W], rbr[:]
                                    )
                            ctxts_all[p][j] = ctxt
                        # pacing cost keeps filler flowing through the
                        # normalize chain's PE-idle window
                        return (2500.0, fn)

                    # scores run one chunk ahead of PV
                    ops.append(mk_sc(c=0))
                    for c in range(1, nch):
                        ops.append(mk_sc(c=c))
                        ops.append(mk_pv(c=c - 1))
                    ops.append(mk_pv(c=nch - 1))
                    ops.append(mk_norm())
                return ops

            def weave(primary, filler):
                # insert due filler BEFORE each primary item so wait-bound
                # attention matmuls have ready PE work queued ahead of them
                tp = sum(c for c, _ in primary) or 1.0
                tf = sum(c for c, _ in filler)
                done_p = 0.0
                done_f = 0.0
                fi = 0
                for c, fn in primary:
                    done_p += c
                    while fi < len(filler) and done_f < tf * (done_p / tp) - 1e-9:
                        cf, ff = filler[fi]
                        ff()
                        done_f += cf
                        fi += 1
                    fn()
                for cf, ff in filler[fi:]:
                    ff()

            # ---------- program body ----------
            # piece 0 QKV runs alone, then bundles:
            #   bundle p: attention(p) woven with QKV(p+1) + outproj(p-1)
            for _, fn in qkv_filler(0):
                fn()
            for p in range(NPIECE):
                filler = []
                reserve = []
                if p >= 1:
                    opf = op_filler(p - 1)
                    if p == NPIECE - 1:
                        # hold back a few groups to cover the last pair's
                        # normalize chain after the bundle
                        filler += opf[:5]
                        reserve = opf[5:]
                    else:
                        filler += opf
                if p + 1 < NPIECE:
                    filler += qkv_filler(p + 1)
                weave(attn_ops(p), filler)
                for _, fn in reserve:
                    fn()
            for _, fn in op_filler(NPIECE - 1):
                fn()

    nc.compile()
    return nc


_program = None
last_results = None


def _get_program():
    global _program
    if _program is None:
        _program = build_program()
    return _program


def kernel(x, Wq, Wk, Wv, Wo, bo):
    global last_results
    x = np.asarray(x, dtype=np.float32)
    Wq = np.asarray(Wq, dtype=np.float32)
    Wk = np.asarray(Wk, dtype=np.float32)
    Wv = np.asarray(Wv, dtype=np.float32)
    Wo = np.asarray(Wo, dtype=np.float32)
    bo = np.asarray(bo, dtype=np.float32)

    import ml_dtypes
    maskA = np.triu(np.ones((P, P), dtype=ml_dtypes.bfloat16))
    ones = np.ones((P, 64), dtype=np.float32)

    nc = _get_program()
    in_maps = []
    for c in range(8):
        b, hg = c // 2, c % 2
        in_maps.append(
            {
                "xT": np.ascontiguousarray(x[b].T).astype(ml_dtypes.bfloat16),
                "wq": np.ascontiguousarray(
                    Wq[:, DH * hg : DH * hg + DH]
                ).astype(ml_dtypes.bfloat16),
                "wk": np.ascontiguousarray(
                    Wk[:, DH * hg : DH * hg + DH]
                ).astype(ml_dtypes.bfloat16),
                "wv": np.ascontiguousarray(
                    Wv[:, DH * hg : DH * hg + DH]
                ).astype(ml_dtypes.bfloat16),
                "wo": np.ascontiguousarray(Wo[DH * hg : DH * hg + DH, :]),
                "maskA": maskA,
                "ones": ones,
            }
        )
    trace = bool(os.environ.get("KERNEL_TRACE"))
    last_results = run_bass_kernel_spmd(
        nc, in_maps, core_ids=list(range(8)), trace=trace
    )
    outs = [r["out"] for r in last_results.results]
    return np.stack([outs[2 * b] + outs[2 * b + 1] + bo for b in range(4)])


# revision 30
# speedup vs baseline: 1.2274x; 1.0343x over previous
"""Multi-head causal attention (B=4, S=2048, D=1024, H=16, HD=64) on 8 TRN2 cores.

Sharding: core c handles (batch b = c//2, head-group hg = c%2 of 8 heads).
Each core computes QKV projections for its 512-dim head slice, transposed-layout
causal attention, and a partial output projection. Host sums the two head-group
partials per batch and adds the bias.

Per-core pipeline (v2 — software-pipelined across pieces):
  - x fed pre-transposed: xT [1024, 2048]; QT/KT/V via bf16 K=128 matmuls.
  - KT per pair stored bf16 [128, S]: head-even dims on partitions 0-63,
    head-odd on 64-127 (no zero padding); scores run as K=64 matmuls with
    base-partition-64 operands for the odd head (tile_position inferred).
  - Exact causal ragged windows SKS=[0,128,256,384]; a single upper-triangular
    maskA multiplies the 128-wide diagonal block of each kept chunk.
  - P^T = exp(ST/8) on ScalarE; ctx^T = V_aug^T @ P^T with V_aug = [V_h | 1]
    (M=65): row 64 accumulates softmax denominators r[q] for free.
  - normalize: r row copied to SBUF (f32r), broadcast to partitions 0-63 by a
    K=1 f32r matmul (213ns) into a st-ring PSUM tile, reciprocal + muls on
    DVE; head-odd ctx rows moved to partitions 64-127 by one SBUF-SBUF DMA
    issued via GpSimd's software DGE.
  - out[s, :] partial = sum_pairs ctxT_pair.T @ Wo_pair (K=128, moving f32r).
  - Engine schedule: QKV matmuls of piece p+1 and the output projection of
    piece p-1 are woven between attention matmuls of piece p in program order
    so TensorE never starves while ScalarE runs the exp chain.
"""

import os
import numpy as np

import concourse.bass as bass
import concourse.mybir as mybir
from concourse import bacc
import concourse.tile as tile
from concourse.bass_utils import run_bass_kernel_spmd

F32 = mybir.dt.float32
F32R = mybir.dt.float32r
BF16 = mybir.dt.bfloat16
EXP = mybir.ActivationFunctionType.Exp

P = 128
S = 2048
DIN = 1024
DH = 512          # per-core d_out slice (8 heads x 64)
NKC = DIN // P    # 8 contraction chunks
NPAIR = 4         # head pairs per core
NPIECE = S // 512 # 4 q pieces
W = 512

# exact causal ragged start offsets for diagonal chunks (krel = chunk - 4*piece)
SKS = [0, 128, 256, 384]

MM_NS = 0.4167  # PE ns/col, used only for weave pacing


def build_program() -> bass.Bass:
    nc = bacc.Bacc("TRN2", target_bir_lowering=False)

    xT_d = nc.dram_tensor("xT", [DIN, S], BF16, kind="ExternalInput")
    wq_d = nc.dram_tensor("wq", [DIN, DH], BF16, kind="ExternalInput")
    wk_d = nc.dram_tensor("wk", [DIN, DH], BF16, kind="ExternalInput")
    wv_d = nc.dram_tensor("wv", [DIN, DH], BF16, kind="ExternalInput")
    wo_d = nc.dram_tensor("wo", [DH, DIN], F32, kind="ExternalInput")
    maskA_d = nc.dram_tensor("maskA", [P, P], BF16, kind="ExternalInput")
    ones_d = nc.dram_tensor("ones", [P, 64], F32, kind="ExternalInput")
    out_d = nc.dram_tensor("out", [S, DIN], F32, kind="ExternalOutput")

    with tile.TileContext(nc) as tc:
        with (
            tc.tile_pool(name="consts", bufs=1) as consts,
            tc.tile_pool(name="xtp", bufs=4) as xtp,
            tc.tile_pool(name="qtp", bufs=2) as qtp,
            tc.tile_pool(name="ptp", bufs=4) as ptp,
            tc.tile_pool(name="ctxtp", bufs=2) as ctxtp,
            tc.tile_pool(name="rp", bufs=2) as rp,
            tc.tile_pool(name="osbp", bufs=6) as osbp,
            tc.tile_pool(name="ps_st", bufs=2, space="PSUM") as ps_st,
            tc.tile_pool(name="ps_ctx", bufs=1, space="PSUM") as ps_ctx,
            tc.tile_pool(name="ps_mm", bufs=2, space="PSUM") as ps_mm,
        ):
            # ---- input/weight DMAs, batched one per tensor, issued on SP ----
            xT_r = xT_d.rearrange("(kc p) s -> p kc s", p=P)
            xts = []
            wq_sb = consts.tile([P, NKC, DH], BF16)
            wk_sb = consts.tile([P, NKC, DH], BF16)
            wv_sb = consts.tile([P, NKC, DH], BF16)
            wo_sb = consts.tile([P, NPAIR, DIN], F32R)
            maskA = consts.tile([P, P], BF16)

            wq_r = wq_d.rearrange("(kc p) d -> p kc d", p=P)
            nc.sync.dma_start(
                wq_sb[:, :, 0:P], wq_r[:, :, 0:P]
            )
            # piece-0 x in two half-tiles so the first Q matmuls start sooner
            xt0a = xtp.tile([P, NKC // 2, W], BF16, tag="xta", name="xt0a")
            xt0b = xtp.tile([P, NKC // 2, W], BF16, tag="xtb", name="xt0b")
            nc.sync.dma_start(xt0a[:], xT_r[:, 0 : NKC // 2, 0:W])
            nc.sync.dma_start(xt0b[:], xT_r[:, NKC // 2 : NKC, 0:W])
            xts.append((xt0a, xt0b))
            for j in range(1, NPAIR):
                nc.sync.dma_start(
                    wq_sb[:, :, P * j : P * j + P], wq_r[:, :, P * j : P * j + P]
                )
            nc.sync.dma_start(wv_sb[:], wv_d.rearrange("(kc p) d -> p kc d", p=P))
            nc.sync.dma_start(wk_sb[:], wk_d.rearrange("(kc p) d -> p kc d", p=P))
            for p in range(1, NPIECE):
                xa = xtp.tile([P, NKC // 2, W], BF16, tag="xta", name=f"xt{p}a")
                xb = xtp.tile([P, NKC // 2, W], BF16, tag="xtb", name=f"xt{p}b")
                nc.sync.dma_start(xa[:], xT_r[:, 0 : NKC // 2, W * p : W * p + W])
                nc.sync.dma_start(xb[:], xT_r[:, NKC // 2 : NKC, W * p : W * p + W])
                xts.append((xa, xb))
            nc.sync.dma_start(
                wo_sb[:], wo_d.rearrange("(g p) d -> p g d", p=P).bitcast(F32R)
            )
            nc.sync.dma_start(maskA[:], maskA_d[:])
            ones_r = consts.tile([P, 64], F32R)
            nc.sync.dma_start(ones_r[:], ones_d[:].bitcast(F32R))

            def xt_ap(q, kc):
                half, k = divmod(kc, NKC // 2)
                return xts[q][half][:, k, :]

            # KT per pair [128, S]: head-even dims on partitions 0-63,
            # head-odd on 64-127.  V per piece [128, s-chunk(4), head(8), 65]
            # with ones in column 64 (softmax denominator rides the PV matmul).
            kt_sb = [consts.tile([P, S], BF16, name=f"kt{j}") for j in range(NPAIR)]
            v_sb = [
                consts.tile([P, 4, 8, 65], BF16, name=f"v{pp}") for pp in range(NPIECE)
            ]
            for pp in range(NPIECE):
                nc.vector.memset(v_sb[pp][:, :, :, 64], 1.0)

            qts_all = [[None] * NPAIR for _ in range(NPIECE)]
            ctxts_all = [[None] * NPAIR for _ in range(NPIECE)]

            # ---------- emitters (closures for the weave) ----------
            def q_grp(q, j):
                def fn(q=q, j=j):
                    ps = ps_mm.tile([P, W], F32, tag="mm")
                    for kc in range(NKC):
                        nc.tensor.matmul(
                            ps[:],
                            wq_sb[:, kc, P * j : P * j + P],
                            xt_ap(q, kc),
                            start=(kc == 0),
                            stop=(kc == NKC - 1),
                        )
                    qt = qtp.tile([P, W], BF16, tag=f"qt{j}", name=f"qt{j}_{q}")
                    nc.scalar.copy(qt[:], ps[:])
                    qts_all[q][j] = qt
                return (NKC * W * MM_NS, fn)

            def k_grp(q, j):
                def fn(q=q, j=j):
                    ps = ps_mm.tile([P, W], F32, tag="mm")
                    for kc in range(NKC):
                        nc.tensor.matmul(
                            ps[:],
                            wk_sb[:, kc, P * j : P * j + P],
                            xt_ap(q, kc),
                            start=(kc == 0),
                            stop=(kc == NKC - 1),
                        )
                    nc.vector.tensor_copy(
                        kt_sb[j][:, W * q : W * q + W], ps[:]
                    )
                return (NKC * W * MM_NS, fn)

            def v_grp(q, i):
                def fn(q=q, i=i):
                    ps = ps_mm.tile([P, W], F32, tag="mm")
                    for kc in range(NKC):
                        nc.tensor.matmul(
                            ps[:],
                            xt_ap(q, kc)[:, P * i : P * i + P],
                            wv_sb[:, kc, :],
                            start=(kc == 0),
                            stop=(kc == NKC - 1),
                        )
                    nc.vector.tensor_copy(
                        v_sb[q][:, i, :, 0:64],
                        ps[:].rearrange("q (h d) -> q h d", h=8),
                    )
                return (NKC * W * MM_NS, fn)

            def op_grp(p, si, nsl, st_half=None):
                def fn(p=p, si=si, nsl=nsl, st_half=st_half):
                    if st_half is not None:
                        st_t, half = st_half
                        ps = st_t[:, W * half : W * half + W]
                    else:
                        ps_t = ps_mm.tile([P, W], F32, tag="mm", name=f"op{p}_{si}_{nsl}")
                        ps = ps_t[:]
                    for g in range(NPAIR):
                        nc.tensor.matmul(
                            ps,
                            ctxts_all[p][g][:, P * si : P * si + P],
                            wo_sb[:, g, W * nsl : W * nsl + W],
                            start=(g == 0),
                            stop=(g == NPAIR - 1),
                            skip_group_check=True,
                        )
                    osb = osbp.tile([P, W], F32, tag="osb")
                    nc.vector.tensor_copy(osb[:], ps)
                    nc.sync.dma_start(
                        out_d[
                            W * p + P * si : W * p + P * si + P,
                            W * nsl : W * nsl + W,
                        ],
                        osb[:],
                    )
                return (NPAIR * W * MM_NS, fn)

            def op_filler(p):
                return [op_grp(p, si, nsl) for si in range(4) for nsl in range(2)]

            # ---------- attention primary stream for piece p ----------
            # k_embed: {pair_j: (after_chunk, (cost, fn))} — K-projection
            # groups inserted right after pair j's last kt read so the kt
            # write's WAR wait resolves immediately.
            def attn_ops(p, k_embed=None, sc_idx=None):
                ops = []
                if sc_idx is None:
                    sc_idx = {}
                nch = 4 * p + 4
                for j in range(NPAIR):
                    ctx_holder = {}
                    pts = {}

                    def mk_sc(p=p, j=j, c=0, pts=pts):
                        def fn(p=p, j=j, c=c, pts=pts):
                            krel = c - 4 * p
                            sk = SKS[krel] if krel >= 0 else 0
                            st = ps_st.tile(
                                [P, 2 * W], F32, tag="st", name=f"st{p}_{j}_{c}"
                            )
                            for hl in range(2):
                                nc.tensor.matmul(
                                    st[:, W * hl + sk : W * hl + W],
                                    kt_sb[j][64 * hl : 64 * hl + 64, P * c : P * c + P],
                                    qts_all[p][j][64 * hl : 64 * hl + 64, sk:W],
                                    start=True,
                                    stop=True,
                                )
                            pt = ptp.tile(
                                [P, 2 * W], BF16, tag="pt", name=f"pt{p}_{j}_{c}"
                            )
                            st3 = st[:].rearrange("q (h n) -> q h n", h=2)
                            pt3 = pt[:].rearrange("q (h n) -> q h n", h=2)
                            nc.scalar.activation(
                                pt3[:, :, sk:W], st3[:, :, sk:W], EXP, scale=0.125
                            )
                            if krel >= 0:
                                nc.vector.tensor_mul(
                                    pt3[:, :, sk : sk + P],
                                    pt3[:, :, sk : sk + P],
                                    maskA[:, None, :].broadcast_to([P, 2, P]),
                                )
                            pts[c] = pt
                        krel = c - 4 * p
                        sk = SKS[krel] if krel >= 0 else 0
                        return (2 * (W - sk) * MM_NS, fn)

                    def mk_pv(p=p, j=j, c=0, pts=pts, ctx_holder=ctx_holder, nch=nch):
                        def fn(p=p, j=j, c=c, pts=pts, ctx_holder=ctx_holder, nch=nch):
                            krel = c - 4 * p
                            sk = SKS[krel] if krel >= 0 else 0
                            if c == 0:
                                ctx_holder["ctx"] = ps_ctx.tile(
                                    [P, 2 * W], F32, tag="ctx", name=f"ctx{p}_{j}"
                                )
                            ctx = ctx_holder["ctx"]
                            pt = pts.pop(c)
                            for hl in range(2):
                                h = 2 * j + hl
                                nc.tensor.matmul(
                                    ctx[0:65, W * hl + sk : W * hl + W],
                                    v_sb[c // 4][:, c % 4, h, :],
                                    pt[:, W * hl + sk : W * hl + W],
                                    start=(c == 0),
                                    stop=(c == nch - 1),
                                    skip_group_check=True,
                                )
                        krel = c - 4 * p
                        sk = SKS[krel] if krel >= 0 else 0
                        return (2 * (W - sk) * MM_NS, fn)

                    def mk_norm(p=p, j=j, ctx_holder=ctx_holder):
                        def fn(p=p, j=j, ctx_holder=ctx_holder):
                            # r row (partition 64) -> SBUF f32r, broadcast to
                            # partitions 0-63 with a K=1 f32r matmul into a
                            # st-ring PSUM tile, reciprocal, then normalize.
                            # hl=1 first so its SBUF-SBUF move starts early.
                            ctx = ctx_holder["ctx"]
                            ctxt = ctxtp.tile(
                                [P, W], F32R, tag=f"ctxt{j}", name=f"ctxt{j}_{p}"
                            )
                            bc = ps_st.tile(
                                [P, 2 * W], F32, tag="st", name=f"bc{p}_{j}"
                            )
                            for hl in (1, 0):
                                rrow = rp.tile(
                                    [P, W], F32R, tag=f"rr{hl}", name=f"rr{hl}_{p}_{j}"
                                )
                                nc.vector.tensor_copy(
                                    rrow[64:65, :], ctx[64:65, W * hl : W * hl + W]
                                )
                                nc.tensor.matmul(
                                    bc[0:64, W * hl : W * hl + W],
                                    ones_r[64:65, :],
                                    rrow[64:65, :],
                                    start=True,
                                    stop=True,
                                    skip_group_check=True,
                                )
                                rbr = rp.tile(
                                    [64, W], F32, tag=f"rb{hl}", name=f"rb{hl}_{p}_{j}"
                                )
                                nc.vector.reciprocal(
                                    rbr[:], bc[0:64, W * hl : W * hl + W]
                                )
                                if hl == 1:
                                    hst = rp.tile(
                                        [64, W], F32R, tag="hst", name=f"hs{p}_{j}"
                                    )
                                    nc.vector.tensor_mul(
                                        hst[:], ctx[0:64, W : 2 * W], rbr[:]
                                    )
                                    nc.gpsimd.dma_start(ctxt[64:128, :], hst[:])
                                else:
                                    nc.vector.tensor_mul(
                                        ctxt[0:64, :], ctx[0:64, 0:W], rbr[:]
                                    )
                            ctxts_all[p][j] = ctxt
                        # pacing cost keeps filler flowing through the
                        # normalize chain's PE-idle window
                        return (2500.0, fn)

                    # scores run one chunk ahead of PV
                    ke = k_embed.get(j) if k_embed else None
                    ops.append(mk_sc(c=0))
                    if ke and ke[0] == 0:
                        ops.append(ke[1])
                    for c in range(1, nch):
                        sc_idx[(j, c)] = len(ops)
                        ops.append(mk_sc(c=c))
                        if ke and ke[0] == c:
                            ops.append(ke[1])
                        ops.append(mk_pv(c=c - 1))
                    ops.append(mk_pv(c=nch - 1))
                    ops.append(mk_norm())
                # trailing virtual cost: under-pace the filler slightly so a
                # few ready items remain to cover the bundle-boundary
                # normalize chain
                ops.append((3000.0, lambda: None))
                return ops

            def weave(primary, filler, deadlines=None):
                # insert due filler BEFORE each primary item so wait-bound
                # attention matmuls have ready PE work queued ahead of them.
                # deadlines[i] (optional) = primary index before which
                # filler[i] MUST be emitted (correctness, not pacing).
                tp = sum(c for c, _ in primary) or 1.0
                tf = sum(c for c, _ in filler)
                done_p = 0.0
                done_f = 0.0
                fi = 0
                for pi, (c, fn) in enumerate(primary):
                    done_p += c
                    while fi < len(filler) and (
                        done_f < tf * (done_p / tp) - 1e-9
                        or (
                            deadlines is not None
                            and deadlines[fi] is not None
                            and deadlines[fi] <= pi
                        )
                    ):
                        cf, ff = filler[fi]
                        ff()
                        done_f += cf
                        fi += 1
                    fn()
                for cf, ff in filler[fi:]:
                    ff()

            # ---------- program body ----------
            # pre: QKV(0); bundles:
            #   B0: attn(0) + [Q1 V1] + K1 embedded after pair j's last sc
            #   B1: attn(1) + [op0 Q2 V2] + K2 embedded
            #   B2: attn(2) + [op1 Q3]
            #   B3: attn(3) + [V3 op2] + K3 embedded after pair j's sc(11)
            #   tail: op(3), first groups on the idle st ring for depth
            for it in (
                [q_grp(0, j) for j in range(NPAIR)]
                + [v_grp(0, i) for i in range(4)]
                + [k_grp(0, j) for j in range(NPAIR)]
            ):
                it[1]()

            for p in range(NPIECE):
                if p == 0:
                    k_embed = {j: (3, k_grp(1, j)) for j in range(NPAIR)}
                elif p == 1:
                    k_embed = {j: (7, k_grp(2, j)) for j in range(NPAIR)}
                elif p == 2:
                    k_embed = None
                else:
                    # K(3)-j embedded after pair j's sc(11): old chunks don't
                    # falsely wait on the kt write, the diagonal follows it.
                    k_embed = {j: (11, k_grp(3, j)) for j in range(NPAIR)}
                sc_idx = {}
                primary = attn_ops(p, k_embed, sc_idx)
                filler = []
                fdl = []

                def add(item, dl=None):
                    filler.append(item)
                    fdl.append(dl)

                # same-bundle V consumers get hard deadlines (program-order
                # correctness); everything else is paced freely.
                if p == 2:
                    for i in range(4):
                        add(v_grp(2, i), sc_idx[(0, 8 + i)])
                elif p == 3:
                    for i in range(4):
                        add(v_grp(3, i), sc_idx[(0, 12 + i)])
                if p >= 1:
                    for it in op_filler(p - 1):
                        add(it)
                if p == 0:
                    for j in range(NPAIR):
                        add(q_grp(1, j))
                    for i in range(4):
                        add(v_grp(1, i))
                elif p == 1:
                    for j in range(NPAIR):
                        add(q_grp(2, j))
                elif p == 2:
                    for j in range(NPAIR):
                        add(q_grp(3, j))
                weave(primary, filler, fdl)

            # final outproj: 4 groups ride the now-idle st ring (deeper
            # PSUM pipeline through the last normalize chain), rest on mm
            groups = [(si, nsl) for si in range(4) for nsl in range(2)]
            for idx in range(0, 4, 2):
                st_t = ps_st.tile([P, 2 * W], F32, tag="st", name=f"opst{idx}")
                for half in range(2):
                    si, nsl = groups[idx + half]
                    op_grp(NPIECE - 1, si, nsl, st_half=(st_t, half))[1]()
            for si, nsl in groups[4:]:
                op_grp(NPIECE - 1, si, nsl)[1]()

    nc.compile()
    return nc


_program = None
last_results = None


def _get_program():
    global _program
    if _program is None:
        _program = build_program()
    return _program


def kernel(x, Wq, Wk, Wv, Wo, bo):
    global last_results
    x = np.asarray(x, dtype=np.float32)
    Wq = np.asarray(Wq, dtype=np.float32)
    Wk = np.asarray(Wk, dtype=np.float32)
    Wv = np.asarray(Wv, dtype=np.float32)
    Wo = np.asarray(Wo, dtype=np.float32)
    bo = np.asarray(bo, dtype=np.float32)

    import ml_dtypes
    maskA = np.triu(np.ones((P, P), dtype=ml_dtypes.bfloat16))
    ones = np.ones((P, 64), dtype=np.float32)

    nc = _get_program()
    in_maps = []
    for c in range(8):
        b, hg = c // 2, c % 2
        in_maps.append(
            {
                "xT": np.ascontiguousarray(x[b].T).astype(ml_dtypes.bfloat16),
                "wq": np.ascontiguousarray(
                    Wq[:, DH * hg : DH * hg + DH]
                ).astype(ml_dtypes.bfloat16),
                "wk": np.ascontiguousarray(
                    Wk[:, DH * hg : DH * hg + DH]
                ).astype(ml_dtypes.bfloat16),
                "wv": np.ascontiguousarray(
                    Wv[:, DH * hg : DH * hg + DH]
                ).astype(ml_dtypes.bfloat16),
                "wo": np.ascontiguousarray(Wo[DH * hg : DH * hg + DH, :]),
                "maskA": maskA,
                "ones": ones,
            }
        )
    trace = bool(os.environ.get("KERNEL_TRACE"))
    last_results = run_bass_kernel_spmd(
        nc, in_maps, core_ids=list(range(8)), trace=trace
    )
    outs = [r["out"] for r in last_results.results]
    return np.stack([outs[2 * b] + outs[2 * b + 1] + bo for b in range(4)])


# revision 33
# speedup vs baseline: 1.2319x; 1.0037x over previous
"""Multi-head causal attention (B=4, S=2048, D=1024, H=16, HD=64) on 8 TRN2 cores.

Sharding: core c handles (batch b = c//2, head-group hg = c%2 of 8 heads).
Each core computes QKV projections for its 512-dim head slice, transposed-layout
causal attention, and a partial output projection. Host sums the two head-group
partials per batch and adds the bias.

Per-core pipeline (v2 — software-pipelined across pieces):
  - x fed pre-transposed: xT [1024, 2048]; QT/KT/V via bf16 K=128 matmuls.
  - KT per pair stored bf16 [128, S]: head-even dims on partitions 0-63,
    head-odd on 64-127 (no zero padding); scores run as K=64 matmuls with
    base-partition-64 operands for the odd head (tile_position inferred).
  - Exact causal ragged windows SKS=[0,128,256,384]; a single upper-triangular
    maskA multiplies the 128-wide diagonal block of each kept chunk.
  - P^T = exp(ST/8) on ScalarE; ctx^T = V_aug^T @ P^T with V_aug = [V_h | 1]
    (M=65): row 64 accumulates softmax denominators r[q] for free.
  - normalize: r row copied to SBUF (f32r), broadcast to partitions 0-63 by a
    K=1 f32r matmul (213ns) into a st-ring PSUM tile, reciprocal + muls on
    DVE; head-odd ctx rows moved to partitions 64-127 by one SBUF-SBUF DMA
    issued via GpSimd's software DGE.
  - out[s, :] partial = sum_pairs ctxT_pair.T @ Wo_pair (K=128, moving f32r).
  - Engine schedule: QKV matmuls of piece p+1 and the output projection of
    piece p-1 are woven between attention matmuls of piece p in program order
    so TensorE never starves while ScalarE runs the exp chain.
"""

import os
import numpy as np

import concourse.bass as bass
import concourse.mybir as mybir
from concourse import bacc
import concourse.tile as tile
from concourse.bass_utils import run_bass_kernel_spmd

F32 = mybir.dt.float32
F32R = mybir.dt.float32r
BF16 = mybir.dt.bfloat16
EXP = mybir.ActivationFunctionType.Exp

P = 128
S = 2048
DIN = 1024
DH = 512          # per-core d_out slice (8 heads x 64)
NKC = DIN // P    # 8 contraction chunks
NPAIR = 4         # head pairs per core
NPIECE = S // 512 # 4 q pieces
W = 512

# exact causal ragged start offsets for diagonal chunks (krel = chunk - 4*piece)
SKS = [0, 128, 256, 384]

MM_NS = 0.4167  # PE ns/col, used only for weave pacing


def build_program() -> bass.Bass:
    nc = bacc.Bacc("TRN2", target_bir_lowering=False)

    xT_d = nc.dram_tensor("xT", [DIN, S], BF16, kind="ExternalInput")
    wq_d = nc.dram_tensor("wq", [DIN, DH], BF16, kind="ExternalInput")
    wk_d = nc.dram_tensor("wk", [DIN, DH], BF16, kind="ExternalInput")
    wv_d = nc.dram_tensor("wv", [DIN, DH], BF16, kind="ExternalInput")
    wo_d = nc.dram_tensor("wo", [DH, DIN], F32, kind="ExternalInput")
    maskA_d = nc.dram_tensor("maskA", [P, P], BF16, kind="ExternalInput")
    ones_d = nc.dram_tensor("ones", [P, 64], F32, kind="ExternalInput")
    out_d = nc.dram_tensor("out", [S, DIN], F32, kind="ExternalOutput")

    with tile.TileContext(nc) as tc:
        with (
            tc.tile_pool(name="consts", bufs=1) as consts,
            tc.tile_pool(name="xtp", bufs=4) as xtp,
            tc.tile_pool(name="qtp", bufs=3) as qtp,
            tc.tile_pool(name="ptp", bufs=6) as ptp,
            tc.tile_pool(name="ctxtp", bufs=2) as ctxtp,
            tc.tile_pool(name="rp", bufs=3) as rp,
            tc.tile_pool(name="osbp", bufs=6) as osbp,
            tc.tile_pool(name="ps_st", bufs=2, space="PSUM") as ps_st,
            tc.tile_pool(name="ps_ctx", bufs=1, space="PSUM") as ps_ctx,
            tc.tile_pool(name="ps_mm", bufs=2, space="PSUM") as ps_mm,
        ):
            # ---- input/weight DMAs, batched one per tensor, issued on SP ----
            xT_r = xT_d.rearrange("(kc p) s -> p kc s", p=P)
            xts = []
            wq_sb = consts.tile([P, NKC, DH], BF16)
            wk_sb = consts.tile([P, NKC, DH], BF16)
            wv_sb = consts.tile([P, NKC, DH], BF16)
            wo_sb = consts.tile([P, NPAIR, DIN], F32R)
            maskA = consts.tile([P, P], BF16)

            wq_r = wq_d.rearrange("(kc p) d -> p kc d", p=P)
            nc.sync.dma_start(
                wq_sb[:, 0 : NKC // 2, 0:P], wq_r[:, 0 : NKC // 2, 0:P]
            )
            nc.sync.dma_start(
                wq_sb[:, NKC // 2 : NKC, 0:P], wq_r[:, NKC // 2 : NKC, 0:P]
            )
            # piece-0 x in two half-tiles so the first Q matmuls start sooner
            xt0a = xtp.tile([P, NKC // 2, W], BF16, tag="xta", name="xt0a")
            xt0b = xtp.tile([P, NKC // 2, W], BF16, tag="xtb", name="xt0b")
            nc.sync.dma_start(xt0a[:], xT_r[:, 0 : NKC // 2, 0:W])
            nc.sync.dma_start(xt0b[:], xT_r[:, NKC // 2 : NKC, 0:W])
            xts.append((xt0a, xt0b))
            for j in range(1, NPAIR):
                nc.sync.dma_start(
                    wq_sb[:, :, P * j : P * j + P], wq_r[:, :, P * j : P * j + P]
                )
            nc.sync.dma_start(wv_sb[:], wv_d.rearrange("(kc p) d -> p kc d", p=P))
            nc.sync.dma_start(wk_sb[:], wk_d.rearrange("(kc p) d -> p kc d", p=P))
            for p in range(1, NPIECE):
                xa = xtp.tile([P, NKC // 2, W], BF16, tag="xta", name=f"xt{p}a")
                xb = xtp.tile([P, NKC // 2, W], BF16, tag="xtb", name=f"xt{p}b")
                nc.sync.dma_start(xa[:], xT_r[:, 0 : NKC // 2, W * p : W * p + W])
                nc.sync.dma_start(xb[:], xT_r[:, NKC // 2 : NKC, W * p : W * p + W])
                xts.append((xa, xb))
            nc.sync.dma_start(
                wo_sb[:], wo_d.rearrange("(g p) d -> p g d", p=P).bitcast(F32R)
            )
            nc.sync.dma_start(maskA[:], maskA_d[:])
            ones_r = consts.tile([P, 64], F32R)
            nc.sync.dma_start(ones_r[:], ones_d[:].bitcast(F32R))

            def xt_ap(q, kc):
                half, k = divmod(kc, NKC // 2)
                return xts[q][half][:, k, :]

            # KT per pair [128, S]: head-even dims on partitions 0-63,
            # head-odd on 64-127.  V per piece [128, s-chunk(4), head(8), 65]
            # with ones in column 64 (softmax denominator rides the PV matmul).
            kt_sb = [consts.tile([P, S], BF16, name=f"kt{j}") for j in range(NPAIR)]
            v_sb = [
                consts.tile([P, 4, 8, 65], BF16, name=f"v{pp}") for pp in range(NPIECE)
            ]
            for pp in range(NPIECE):
                nc.vector.memset(v_sb[pp][:, :, :, 64], 1.0)

            qts_all = [[None] * NPAIR for _ in range(NPIECE)]
            ctxts_all = [[None] * NPAIR for _ in range(NPIECE)]

            # ---------- emitters (closures for the weave) ----------
            def q_grp(q, j):
                def fn(q=q, j=j):
                    ps = ps_mm.tile([P, W], F32, tag="mm")
                    for kc in range(NKC):
                        nc.tensor.matmul(
                            ps[:],
                            wq_sb[:, kc, P * j : P * j + P],
                            xt_ap(q, kc),
                            start=(kc == 0),
                            stop=(kc == NKC - 1),
                        )
                    qt = qtp.tile([P, W], BF16, tag=f"qt{j}", name=f"qt{j}_{q}")
                    nc.scalar.copy(qt[:], ps[:])
                    qts_all[q][j] = qt
                return (NKC * W * MM_NS, fn)

            def k_grp(q, j):
                def fn(q=q, j=j):
                    ps = ps_mm.tile([P, W], F32, tag="mm")
                    for kc in range(NKC):
                        nc.tensor.matmul(
                            ps[:],
                            wk_sb[:, kc, P * j : P * j + P],
                            xt_ap(q, kc),
                            start=(kc == 0),
                            stop=(kc == NKC - 1),
                        )
                    nc.vector.tensor_copy(
                        kt_sb[j][:, W * q : W * q + W], ps[:]
                    )
                return (NKC * W * MM_NS, fn)

            def v_grp(q, i):
                def fn(q=q, i=i):
                    ps = ps_mm.tile([P, W], F32, tag="mm")
                    for kc in range(NKC):
                        nc.tensor.matmul(
                            ps[:],
                            xt_ap(q, kc)[:, P * i : P * i + P],
                            wv_sb[:, kc, :],
                            start=(kc == 0),
                            stop=(kc == NKC - 1),
                        )
                    nc.vector.tensor_copy(
                        v_sb[q][:, i, :, 0:64],
                        ps[:].rearrange("q (h d) -> q h d", h=8),
                    )
                return (NKC * W * MM_NS, fn)

            def op_grp(p, si, nsl, st_half=None):
                def fn(p=p, si=si, nsl=nsl, st_half=st_half):
                    if st_half is not None:
                        st_t, half = st_half
                        ps = st_t[:, W * half : W * half + W]
                    else:
                        ps_t = ps_mm.tile([P, W], F32, tag="mm", name=f"op{p}_{si}_{nsl}")
                        ps = ps_t[:]
                    for g in range(NPAIR):
                        nc.tensor.matmul(
                            ps,
                            ctxts_all[p][g][:, P * si : P * si + P],
                            wo_sb[:, g, W * nsl : W * nsl + W],
                            start=(g == 0),
                            stop=(g == NPAIR - 1),
                            skip_group_check=True,
                        )
                    osb = osbp.tile([P, W], F32, tag="osb")
                    if st_half is not None or (p == NPIECE - 1 and (si + nsl) % 2):
                        nc.scalar.copy(osb[:], ps)
                    else:
                        nc.vector.tensor_copy(osb[:], ps)
                    nc.sync.dma_start(
                        out_d[
                            W * p + P * si : W * p + P * si + P,
                            W * nsl : W * nsl + W,
                        ],
                        osb[:],
                    )
                return (NPAIR * W * MM_NS, fn)

            def op_filler(p):
                return [op_grp(p, si, nsl) for si in range(4) for nsl in range(2)]

            # ---------- attention primary stream for piece p ----------
            # k_embed: {pair_j: (after_chunk, (cost, fn))} — K-projection
            # groups inserted right after pair j's last kt read so the kt
            # write's WAR wait resolves immediately.
            def attn_ops(p, k_embed=None, sc_idx=None):
                ops = []
                if sc_idx is None:
                    sc_idx = {}
                nch = 4 * p + 4
                for j in range(NPAIR):
                    ctx_holder = {}
                    pts = {}

                    def mk_sc(p=p, j=j, c=0, pts=pts):
                        def fn(p=p, j=j, c=c, pts=pts):
                            krel = c - 4 * p
                            sk = SKS[krel] if krel >= 0 else 0
                            st = ps_st.tile(
                                [P, 2 * W], F32, tag="st", name=f"st{p}_{j}_{c}"
                            )
                            for hl in range(2):
                                nc.tensor.matmul(
                                    st[:, W * hl + sk : W * hl + W],
                                    kt_sb[j][64 * hl : 64 * hl + 64, P * c : P * c + P],
                                    qts_all[p][j][64 * hl : 64 * hl + 64, sk:W],
                                    start=True,
                                    stop=True,
                                )
                            pt = ptp.tile(
                                [P, 2 * W], BF16, tag="pt", name=f"pt{p}_{j}_{c}"
                            )
                            st3 = st[:].rearrange("q (h n) -> q h n", h=2)
                            pt3 = pt[:].rearrange("q (h n) -> q h n", h=2)
                            nc.scalar.activation(
                                pt3[:, :, sk:W], st3[:, :, sk:W], EXP, scale=0.125
                            )
                            if krel >= 0:
                                nc.vector.tensor_mul(
                                    pt3[:, :, sk : sk + P],
                                    pt3[:, :, sk : sk + P],
                                    maskA[:, None, :].broadcast_to([P, 2, P]),
                                )
                            pts[c] = pt
                        krel = c - 4 * p
                        sk = SKS[krel] if krel >= 0 else 0
                        return (2 * (W - sk) * MM_NS, fn)

                    def mk_pv(p=p, j=j, c=0, pts=pts, ctx_holder=ctx_holder, nch=nch):
                        def fn(p=p, j=j, c=c, pts=pts, ctx_holder=ctx_holder, nch=nch):
                            krel = c - 4 * p
                            sk = SKS[krel] if krel >= 0 else 0
                            if c == 0:
                                ctx_holder["ctx"] = ps_ctx.tile(
                                    [P, 2 * W], F32, tag="ctx", name=f"ctx{p}_{j}"
                                )
                            ctx = ctx_holder["ctx"]
                            pt = pts.pop(c)
                            for hl in range(2):
                                h = 2 * j + hl
                                nc.tensor.matmul(
                                    ctx[0:65, W * hl + sk : W * hl + W],
                                    v_sb[c // 4][:, c % 4, h, :],
                                    pt[:, W * hl + sk : W * hl + W],
                                    start=(c == 0),
                                    stop=(c == nch - 1),
                                    skip_group_check=True,
                                )
                        krel = c - 4 * p
                        sk = SKS[krel] if krel >= 0 else 0
                        return (2 * (W - sk) * MM_NS, fn)

                    def mk_norm(p=p, j=j, ctx_holder=ctx_holder):
                        def fn(p=p, j=j, ctx_holder=ctx_holder):
                            # r row (partition 64) -> SBUF f32r, broadcast to
                            # partitions 0-63 with a K=1 f32r matmul into a
                            # st-ring PSUM tile, reciprocal, then normalize.
                            # hl=1 first so its SBUF-SBUF move starts early.
                            ctx = ctx_holder["ctx"]
                            ctxt = ctxtp.tile(
                                [P, W], F32R, tag=f"ctxt{j}", name=f"ctxt{j}_{p}"
                            )
                            bc = ps_st.tile(
                                [P, 2 * W], F32, tag="st", name=f"bc{p}_{j}"
                            )
                            for hl in (1, 0):
                                rrow = rp.tile(
                                    [P, W], F32R, tag=f"rr{hl}", name=f"rr{hl}_{p}_{j}"
                                )
                                nc.vector.tensor_copy(
                                    rrow[64:65, :], ctx[64:65, W * hl : W * hl + W]
                                )
                                nc.tensor.matmul(
                                    bc[0:64, W * hl : W * hl + W],
                                    ones_r[64:65, :],
                                    rrow[64:65, :],
                                    start=True,
                                    stop=True,
                                    skip_group_check=True,
                                )
                                rbr = rp.tile(
                                    [64, W], F32, tag=f"rb{hl}", name=f"rb{hl}_{p}_{j}"
                                )
                                nc.vector.reciprocal(
                                    rbr[:], bc[0:64, W * hl : W * hl + W]
                                )
                                if hl == 1:
                                    hst = rp.tile(
                                        [64, W], F32R, tag="hst", name=f"hs{p}_{j}"
                                    )
                                    nc.vector.tensor_mul(
                                        hst[:], ctx[0:64, W : 2 * W], rbr[:]
                                    )
                                    nc.gpsimd.dma_start(ctxt[64:128, :], hst[:])
                                else:
                                    nc.vector.tensor_mul(
                                        ctxt[0:64, :], ctx[0:64, 0:W], rbr[:]
                                    )
                            ctxts_all[p][j] = ctxt
                        # pacing cost keeps filler flowing through the
                        # normalize chain's PE-idle window
                        return (2500.0, fn)

                    # scores run one chunk ahead of PV
                    ke = k_embed.get(j) if k_embed else None
                    ops.append(mk_sc(c=0))
                    if ke and ke[0] == 0:
                        ops.append(ke[1])
                    for c in range(1, nch):
                        sc_idx[(j, c)] = len(ops)
                        ops.append(mk_sc(c=c))
                        if ke and ke[0] == c:
                            ops.append(ke[1])
                        ops.append(mk_pv(c=c - 1))
                    ops.append(mk_pv(c=nch - 1))
                    ops.append(mk_norm())
                # trailing virtual cost: under-pace the filler slightly so a
                # few ready items remain to cover the bundle-boundary
                # normalize chain
                ops.append((3000.0, lambda: None))
                return ops

            def weave(primary, filler, deadlines=None):
                # insert due filler BEFORE each primary item so wait-bound
                # attention matmuls have ready PE work queued ahead of them.
                # deadlines[i] (optional) = primary index before which
                # filler[i] MUST be emitted (correctness, not pacing).
                tp = sum(c for c, _ in primary) or 1.0
                tf = sum(c for c, _ in filler)
                done_p = 0.0
                done_f = 0.0
                fi = 0
                for pi, (c, fn) in enumerate(primary):
                    done_p += c
                    while fi < len(filler) and (
                        done_f < tf * (done_p / tp) - 1e-9
                        or (
                            deadlines is not None
                            and deadlines[fi] is not None
                            and deadlines[fi] <= pi
                        )
                    ):
                        cf, ff = filler[fi]
                        ff()
                        done_f += cf
                        fi += 1
                    fn()
                for cf, ff in filler[fi:]:
                    ff()

            # ---------- program body ----------
            # pre: QKV(0); bundles:
            #   B0: attn(0) + [Q1 V1] + K1 embedded after pair j's last sc
            #   B1: attn(1) + [op0 Q2 V2] + K2 embedded
            #   B2: attn(2) + [op1 Q3]
            #   B3: attn(3) + [V3 op2] + K3 embedded after pair j's sc(11)
            #   tail: op(3), first groups on the idle st ring for depth
            for it in (
                [q_grp(0, j) for j in range(NPAIR)]
                + [v_grp(0, i) for i in range(4)]
                + [k_grp(0, j) for j in range(NPAIR)]
            ):
                it[1]()

            for p in range(NPIECE):
                if p == 0:
                    k_embed = {j: (3, k_grp(1, j)) for j in range(NPAIR)}
                elif p == 1:
                    k_embed = {j: (7, k_grp(2, j)) for j in range(NPAIR)}
                elif p == 2:
                    k_embed = None
                else:
                    # K(3)-j embedded after pair j's sc(11): old chunks don't
                    # falsely wait on the kt write, the diagonal follows it.
                    k_embed = {j: (11, k_grp(3, j)) for j in range(NPAIR)}
                sc_idx = {}
                primary = attn_ops(p, k_embed, sc_idx)
                filler = []
                fdl = []

                def add(item, dl=None):
                    filler.append(item)
                    fdl.append(dl)

                # same-bundle V consumers get hard deadlines (program-order
                # correctness); everything else is paced freely.
                if p == 2:
                    for i in range(4):
                        add(v_grp(2, i), sc_idx[(0, 8 + i)])
                elif p == 3:
                    for i in range(4):
                        add(v_grp(3, i), sc_idx[(0, 12 + i)])
                if p >= 1:
                    for it in op_filler(p - 1):
                        add(it)
                if p == 0:
                    for j in range(NPAIR):
                        add(q_grp(1, j))
                    for i in range(4):
                        add(v_grp(1, i))
                elif p == 1:
                    for j in range(NPAIR):
                        add(q_grp(2, j))
                elif p == 2:
                    for j in range(NPAIR):
                        add(q_grp(3, j))
                weave(primary, filler, fdl)

            # final outproj: 4 groups ride the now-idle st ring (deeper
            # PSUM pipeline through the last normalize chain), rest on mm
            groups = [(si, nsl) for si in range(4) for nsl in range(2)]
            for idx in range(0, 4, 2):
                st_t = ps_st.tile([P, 2 * W], F32, tag="st", name=f"opst{idx}")
                for half in range(2):
                    si, nsl = groups[idx + half]
                    op_grp(NPIECE - 1, si, nsl, st_half=(st_t, half))[1]()
            for si, nsl in groups[4:]:
                op_grp(NPIECE - 1, si, nsl)[1]()

    nc.compile()
    return nc


_program = None
last_results = None


def _get_program():
    global _program
    if _program is None:
        _program = build_program()
    return _program


def kernel(x, Wq, Wk, Wv, Wo, bo):
    global last_results
    x = np.asarray(x, dtype=np.float32)
    Wq = np.asarray(Wq, dtype=np.float32)
    Wk = np.asarray(Wk, dtype=np.float32)
    Wv = np.asarray(Wv, dtype=np.float32)
    Wo = np.asarray(Wo, dtype=np.float32)
    bo = np.asarray(bo, dtype=np.float32)

    import ml_dtypes
    maskA = np.triu(np.ones((P, P), dtype=ml_dtypes.bfloat16))
    ones = np.ones((P, 64), dtype=np.float32)

    nc = _get_program()
    in_maps = []
    for c in range(8):
        b, hg = c // 2, c % 2
        in_maps.append(
            {
                "xT": np.ascontiguousarray(x[b].T).astype(ml_dtypes.bfloat16),
                "wq": np.ascontiguousarray(
                    Wq[:, DH * hg : DH * hg + DH]
                ).astype(ml_dtypes.bfloat16),
                "wk": np.ascontiguousarray(
                    Wk[:, DH * hg : DH * hg + DH]
                ).astype(ml_dtypes.bfloat16),
                "wv": np.ascontiguousarray(
                    Wv[:, DH * hg : DH * hg + DH]
                ).astype(ml_dtypes.bfloat16),
                "wo": np.ascontiguousarray(Wo[DH * hg : DH * hg + DH, :]),
                "maskA": maskA,
                "ones": ones,
            }
        )
    trace = bool(os.environ.get("KERNEL_TRACE"))
    last_results = run_bass_kernel_spmd(
        nc, in_maps, core_ids=list(range(8)), trace=trace
    )
    outs = [r["out"] for r in last_results.results]
    return np.stack([outs[2 * b] + outs[2 * b + 1] + bo for b in range(4)])
